# revision 1
# baseline (speedup 1.0000x reference)
"""Kernel-target-alignment loss on 8 TRN2 NeuronCores.

Math: Xs = X*sqrt(params); d2_ij = ||Xs_i - Xs_j||^2; K = exp(-d2) (diag == 1);
kta = sum(K*tt^T) / (N*sqrt(sum(K*K)));  return -kta.

Strategy (row-sharded across 8 cores, 1024 rows each):
  A_ij = 2*sum_d p_d x_i x_j - sq_i - sq_j  (= -d2), built per [128,1024] tile as
  one augmented fp32 matmul (K=65: [2p*X^T ; ones] x [X^T ; -sq]) giving
  2G - sq_j in PSUM; the -sq_i row term rides the ACT exp bias (a [128,1] column
  obtained by a K=1 PE matmul "transpose" of the -sq row, so row and column sq
  are bit-identical).  E = exp(A) in bf16.  Two fused DVE tensor_tensor_reduce
  ops per tile give row-sums of E*E (for sum K^2) and E*t_col (for t^T K t).
  Per-core partials return to the host for the final scalar combine.  No sqrt
  anywhere (lhs carries p*x, rhs carries x), so A_ii is fp32-exact ~0 and
  bf16(exp(A_ii)) == 1.0 exactly, matching the reference's unit diagonal.
"""

import numpy as np

import concourse.bass as bass
import concourse.bacc as bacc
import concourse.tile as tile
import concourse.mybir as mybir
from concourse.bass_utils import run_bass_kernel_spmd

N = 8192
D = 64
NCORES = 8
RPC = N // NCORES          # 1024 rows per core
NRB = RPC // 128           # 8 row blocks of 128 rows
CW = 1024                  # column tile width (2 PSUM banks fp32)
NCT = N // CW              # 8 column tiles
NSLOT = NRB * NCT          # 64 accumulator slots per core

F32 = mybir.dt.float32
BF16 = mybir.dt.bfloat16


def _ap(tensor, ap):
    return bass.AP(tensor=tensor, offset=0, ap=ap)


def build_kernel(variant="stt"):
    nc = bacc.Bacc("TRN2", target_bir_lowering=False)

    xt_d = nc.dram_tensor("xt", [D, N], F32, kind="ExternalInput")
    lt_d = nc.dram_tensor("lt", [D, RPC], F32, kind="ExternalInput")
    t_d = nc.dram_tensor("t", [N], F32, kind="ExternalInput")
    params_d = nc.dram_tensor("params", [D], F32, kind="ExternalInput")
    rsq_d = nc.dram_tensor("rsq_scratch", [RPC], F32)
    s1o_d = nc.dram_tensor("s1o", [128, NSLOT], F32, kind="ExternalOutput")
    s2o_d = nc.dram_tensor("s2o", [128, NSLOT], F32, kind="ExternalOutput")

    with tile.TileContext(nc) as tc:
        with (
            tc.tile_pool(name="const", bufs=1) as cpool,
            tc.tile_pool(name="ztiles", bufs=3) as zpool,
            tc.tile_pool(name="etile", bufs=4) as epool,
            tc.tile_pool(name="scratch", bufs=4) as spool,
            tc.tile_pool(name="mmpsum", bufs=2, space="PSUM") as mpool,
            tc.tile_pool(name="setpsum", bufs=3, space="PSUM") as qpool,
        ):
            # ---- persistent SBUF tensors -------------------------------------
            R = cpool.tile([D + 1, N], F32, tag="R")       # [x^T ; -sq]
            L = cpool.tile([D + 1, RPC], F32, tag="L")     # local [2p*x^T ; ones]
            lt_sb = cpool.tile([D, RPC], F32, tag="ltsb")  # local x^T slice
            sqloc = cpool.tile([1, RPC], F32, tag="sqloc")  # local -sq row
            xs1l = cpool.tile([D, RPC], F32, tag="xs1l")   # local p*x^T
            xs1 = cpool.tile([D, N], F32, tag="xs1")       # p * x^T
            tcol = cpool.tile([128, N], BF16, tag="tcol")  # t broadcast to 128 parts
            tcolf = cpool.tile([128, N], F32, tag="tcolf")
            psb = cpool.tile([D, 1], F32, tag="psb")
            p2sb = cpool.tile([D, 1], F32, tag="p2sb")
            neg1 = cpool.tile([D, 1], F32, tag="neg1")
            rsqn = cpool.tile([128, NRB], F32, tag="rsqn")
            rsqn2 = cpool.tile([128, NRB], F32, tag="rsqn2")
            s1acc = cpool.tile([128, NSLOT], F32, tag="s1acc")
            s2acc = cpool.tile([128, NSLOT], F32, tag="s2acc")

            # ---- setup -------------------------------------------------------
            for s in range(16):
                sl = slice(s * 512, (s + 1) * 512)
                nc.sync.dma_start(out=R[0:D, sl], in_=xt_d[:, sl])
            nc.gpsimd.dma_start(out=psb[:, :], in_=_ap(params_d, [[1, D], [0, 1]]))
            nc.sync.dma_start(out=lt_sb[:, :], in_=lt_d[:, :])
            for s in range(8):
                sl = slice(s * (N // 8), (s + 1) * (N // 8))
                nc.sync.dma_start(
                    out=tcolf[:, sl],
                    in_=bass.AP(tensor=t_d, offset=s * (N // 8), ap=[[0, 128], [1, N // 8]]),
                )
            nc.vector.tensor_scalar_mul(p2sb[:, :], psb[:, :], 2.0)
            nc.vector.memset(neg1[:, :], -1.0)
            nc.gpsimd.memset(L[D : D + 1, :], 1.0)
            nc.vector.tensor_scalar_mul(L[0:D, :], lt_sb[:, :], p2sb[:, :])
            nc.vector.tensor_scalar_mul(xs1l[:, :], lt_sb[:, :], psb[:, :])

            # xs1 = p*x^T  (sliced for pipelining)
            for s in range(8):
                sl = slice(s * (N // 8), (s + 1) * (N // 8))
                nc.vector.tensor_scalar_mul(xs1[:, sl], R[0:D, sl], psb[:, :])

            # col-layout -sq (R row D) via PE partition-reduce of z = xs1 * x
            for s in range(16):
                sl = slice(s * 512, (s + 1) * 512)
                zt = zpool.tile([D, 512], F32, tag="z")
                nc.vector.tensor_mul(zt[:, :], xs1[:, sl], R[0:D, sl])
                q = qpool.tile([128, 512], F32, tag="qps")
                nc.tensor.matmul(
                    q[0:1, :], neg1[:, :], zt[:, :], start=True, stop=True
                )
                nc.scalar.copy(out=R[D : D + 1, sl], in_=q[0:1, :])

            # local -sq row for this core's rows (same fp ops as column path)
            for s in range(RPC // 512):
                sl = slice(s * 512, (s + 1) * 512)
                zt = zpool.tile([D, 512], F32, tag="z")
                nc.vector.tensor_mul(zt[:, :], xs1l[:, sl], lt_sb[:, sl])
                q = qpool.tile([128, 512], F32, tag="qps")
                nc.tensor.matmul(
                    q[0:1, :], neg1[:, :], zt[:, :], start=True, stop=True
                )
                nc.scalar.copy(out=sqloc[:, sl], in_=q[0:1, :])

            # row-layout -sq for the exp bias: bounce through DRAM so the
            # [1, RPC] row can be re-read as a [128, NRB] partition-major tile:
            # rsqn[p, rb] = sqloc[0, rb*128 + p]
            nc.gpsimd.dma_start(out=_ap(rsq_d, [[0, 1], [1, RPC]]), in_=sqloc[:, :])
            nc.gpsimd.dma_start(out=rsqn[:, :], in_=_ap(rsq_d, [[1, 128], [128, NRB]]))
            nc.vector.tensor_scalar_mul(rsqn2[:, :], rsqn[:, :], 2.0)

            # tcol: cast broadcast t to bf16
            for s in range(8):
                sl = slice(s * (N // 8), (s + 1) * (N // 8))
                nc.vector.tensor_copy(out=tcol[:, sl], in_=tcolf[:, sl])

            if variant == "nott":
                nc.vector.memset(s1acc[:, :], 0.0)
                nc.vector.memset(s2acc[:, :], 0.0)
            # ---- main loop ---------------------------------------------------
            for rb in range(NRB):
                lhsT = L[:, rb * 128 : (rb + 1) * 128]
                bias = rsqn[:, rb : rb + 1]
                for ct in range(NCT):
                    slot = rb * NCT + ct
                    mm = mpool.tile([128, CW], F32, tag="mm")
                    for j in range(CW // 512):
                        sl = slice(ct * CW + j * 512, ct * CW + (j + 1) * 512)
                        nc.tensor.matmul(
                            mm[:, j * 512 : (j + 1) * 512],
                            lhsT,
                            R[:, sl],
                            start=True,
                            stop=True,
                        )
                    EDT = F32 if variant == "ttrf32" else BF16
                    E = epool.tile([128, CW], EDT, tag="E")
                    if variant == "noexp":
                        nc.scalar.copy(out=E[:, :], in_=mm[:, :])
                    else:
                        nc.scalar.activation(
                            out=E[:, :], in_=mm[:, :],
                            func=mybir.ActivationFunctionType.Exp,
                            bias=bias, scale=1.0,
                        )
                    if variant == "nott":
                        continue
                    sc1 = spool.tile([128, CW], EDT, tag="sc1")
                    tcol_in = tcolf if variant == "ttrf32" else tcol
                    if variant in ("stt", "g1", "act2"):
                        if variant == "g1":
                            nc.gpsimd.scalar_tensor_tensor(
                                out=sc1[:, :], in0=E[:, :], scalar=1.0, in1=E[:, :],
                                op0=mybir.AluOpType.mult, op1=mybir.AluOpType.mult,
                                accum_out=s1acc[:, slot : slot + 1],
                            )
                        elif variant == "act2":
                            nc.scalar.activation(
                                out=sc1[:, :], in_=mm[:, :],
                                func=mybir.ActivationFunctionType.Exp,
                                bias=rsqn2[:, rb : rb + 1], scale=2.0,
                                accum_out=s1acc[:, slot : slot + 1],
                            )
                        else:
                            nc.vector.scalar_tensor_tensor(
                                out=sc1[:, :], in0=E[:, :], scalar=1.0, in1=E[:, :],
                                op0=mybir.AluOpType.mult, op1=mybir.AluOpType.mult,
                                accum_out=s1acc[:, slot : slot + 1],
                            )
                        sc2 = spool.tile([128, CW], EDT, tag="sc2")
                        nc.vector.scalar_tensor_tensor(
                            out=sc2[:, :], in0=E[:, :], scalar=1.0,
                            in1=tcol_in[:, ct * CW : (ct + 1) * CW],
                            op0=mybir.AluOpType.mult, op1=mybir.AluOpType.mult,
                            accum_out=s2acc[:, slot : slot + 1],
                        )
                    else:
                        nc.vector.tensor_tensor_reduce(
                            out=sc1[:, :], in0=E[:, :], in1=E[:, :],
                            scale=1.0, scalar=0.0,
                            op0=mybir.AluOpType.mult, op1=mybir.AluOpType.add,
                            accum_out=s1acc[:, slot : slot + 1],
                        )
                        sc2 = spool.tile([128, CW], EDT, tag="sc2")
                        nc.vector.tensor_tensor_reduce(
                            out=sc2[:, :], in0=E[:, :],
                            in1=tcol_in[:, ct * CW : (ct + 1) * CW],
                            scale=1.0, scalar=0.0,
                            op0=mybir.AluOpType.mult, op1=mybir.AluOpType.add,
                            accum_out=s2acc[:, slot : slot + 1],
                        )

            nc.sync.dma_start(out=s1o_d[:, :], in_=s1acc[:, :])
            nc.sync.dma_start(out=s2o_d[:, :], in_=s2acc[:, :])

    nc.compile()
    return nc


_NC_CACHE = None


def make_in_maps(X, target, params):
    X = np.ascontiguousarray(X, dtype=np.float32)
    target = np.ascontiguousarray(target, dtype=np.float32)
    params = np.ascontiguousarray(params, dtype=np.float32)
    xt = np.ascontiguousarray(X.T)
    return [
        {
            "xt": xt,
            "lt": np.ascontiguousarray(xt[:, c * RPC : (c + 1) * RPC]),
            "t": target,
            "params": params,
        }
        for c in range(NCORES)
    ]


def kernel(X, target, params):
    global _NC_CACHE
    X = np.ascontiguousarray(X, dtype=np.float32)
    target = np.ascontiguousarray(target, dtype=np.float32)
    params = np.ascontiguousarray(params, dtype=np.float32)

    in_maps = make_in_maps(X, target, params)

    if _NC_CACHE is None:
        _NC_CACHE = build_kernel()
    res = run_bass_kernel_spmd(_NC_CACHE, in_maps, core_ids=list(range(NCORES)))

    s1 = 0.0
    s2 = 0.0
    for c in range(NCORES):
        s1o = res.results[c]["s1o"]  # [128, NSLOT]
        s2o = res.results[c]["s2o"]  # [128, NSLOT]
        s1 += float(s1o.sum())
        u = s2o.reshape(128, NRB, NCT).sum(axis=2)              # [128, NRB]
        tb = target[c * RPC : (c + 1) * RPC].reshape(NRB, 128)  # [NRB, 128]
        s2 += float(np.sum(u.T * tb))

    val = -s2 / (N * np.sqrt(s1))
    return np.array(val, dtype=np.float32)



# revision 11
# speedup vs baseline: 1.7726x; 1.7726x over previous
"""Kernel-target-alignment loss on 8 TRN2 NeuronCores (v2: symmetric + bf16).

Math: Xs = X*sqrt(params); d2_ij = ||Xs_i - Xs_j||^2; K = exp(-d2) (diag == 1);
kta = sum(K*tt^T) / (N*sqrt(sum(K*K)));  return -kta.

Design:
  * K is symmetric: compute the 8 diagonal [1024,1024] supertiles (weight 1)
    and the 28 strictly-upper supertiles (weight 2) -- 36/64 of the work.
    Tiles are [128, 1024].  Tiles with column supertile ct=c exist for global
    row blocks rb in [0, 8(c+1)): 8(c+1) of them, so every core takes the
    (c+1) tiles with rb === core (mod 8).  This makes the slot sequence
    (ct pattern, 36 slots) IDENTICAL on every core -- one SPMD program; all
    per-core variation lives in host-packed inputs:
      xlpack [64, 36*128] = xt columns of the core's row block per slot
      tpack  [128, 36]    = target block per slot
  * A = -d2 via one bf16 augmented matmul: lhsT = [XSLp; ones] (K=65),
    rhs = [Xsb; srow] with Xsb = bf16(sqrt(2p)*x), srow_j = bf16(-sq_j) where
    sq comes from the same bf16 products (PE reduce of z = Xsb*Xsb), so the
    ACT exp bias b = -(T + srow) (fp32, from a slot-packed reduce of
    zp = XSLp*XSLp) cancels the diagonal to ~1e-2: bf16 exp(A_ii) == 1 +- 1%.
  * E = exp(A + b) bf16 on ACT.  s1 partial: DVE scalar_tensor_tensor E*E
    with accum -> s1acc[:, slot].  s2 partial: two M=1 PE matmuls
    tpack_slot^T @ E -> PSUM row slot of a [36, 1024] strip, drained once.
  * Host: s1 = sum_slots wgt * sum(s1o[:, slot]);
    s2 = sum_slots wgt * dot(wo[slot], t[ct*1024 : +1024]);
    return -s2 / (N * sqrt(s1)).
"""

import numpy as np

import concourse.bass as bass
import concourse.bacc as bacc
import concourse.tile as tile
import concourse.mybir as mybir
from concourse.bass_utils import run_bass_kernel_spmd

N = 8192
D = 64
NCORES = 8
CW = 1024                  # column tile width
NST = 8                    # supertile grid
NTILES = 36                # tiles (slots) per core
PK = NTILES * 128          # packed lhs columns = 4608

F32 = mybir.dt.float32
BF16 = mybir.dt.bfloat16

# slot pattern: ct per slot, same on every core
SLOT_CT = [c for c in range(NST) for _ in range(c + 1)]
assert len(SLOT_CT) == NTILES


def slot_rbs(core):
    """Global row block per slot for this core."""
    rbs = []
    for c in range(NST):
        for j in range(c + 1):
            rbs.append(8 * j + core)
    return rbs


def slot_weights(core):
    w = []
    for c in range(NST):
        for j in range(c + 1):
            rb = 8 * j + core
            w.append(1.0 if 8 * c <= rb < 8 * (c + 1) else 2.0)
    return w


def _ap(tensor, ap, offset=0):
    return bass.AP(tensor=tensor, offset=offset, ap=ap)


def build_kernel():
    nc = bacc.Bacc("TRN2", target_bir_lowering=False)

    xt_d = nc.dram_tensor("xt", [D, N], F32, kind="ExternalInput")
    xlp_d = nc.dram_tensor("xlp", [D, PK], F32, kind="ExternalInput")
    tp_d = nc.dram_tensor("tp", [128, NTILES], F32, kind="ExternalInput")
    params_d = nc.dram_tensor("params", [D], F32, kind="ExternalInput")
    srowf_d = nc.dram_tensor("srowf_scratch", [N], F32)
    srowb_d = nc.dram_tensor("srowb_scratch", [N], BF16)
    spackf_d = nc.dram_tensor("spackf_scratch", [PK], F32)
    s1o_d = nc.dram_tensor("s1o", [128, NTILES], F32, kind="ExternalOutput")
    wo_d = nc.dram_tensor("wo", [65, 3 * CW], F32, kind="ExternalOutput")

    with tile.TileContext(nc) as tc:
        with (
            tc.tile_pool(name="const", bufs=1) as cpool,
            tc.tile_pool(name="etile", bufs=4) as epool,
            tc.tile_pool(name="scratch", bufs=3) as spool,
            tc.tile_pool(name="mmpsum", bufs=2, space="PSUM") as mpool,
            tc.tile_pool(name="wq", bufs=2, space="PSUM") as wpool,
        ):
            qpool = wpool
            # ---- persistent SBUF tensors -------------------------------------
            xtsb = cpool.tile([D, N], F32, tag="xtsb")
            xlpsb = cpool.tile([D, PK], F32, tag="xlpsb")
            XSR = cpool.tile([D + 1, N], BF16, tag="XSR")    # [Xsb; srow]
            XSLp = cpool.tile([D + 1, PK], BF16, tag="XSLp")  # [Xsb-pack; ones]
            zz = cpool.tile([D, N], BF16, tag="zz")
            zp = cpool.tile([D, PK], BF16, tag="zp")
            psb = cpool.tile([D, 1], F32, tag="psb")
            rp2 = cpool.tile([D, 1], F32, tag="rp2")
            rp = cpool.tile([D, 1], F32, tag="rp")
            negh = cpool.tile([D, 1], BF16, tag="negh")
            srowrowf = cpool.tile([1, N], F32, tag="srowrowf")
            spackrow = cpool.tile([1, PK], F32, tag="spackrow")
            s128f = cpool.tile([128, N // 128], F32, tag="s128f")
            srow128b = cpool.tile([128, N // 128], BF16, tag="srow128b")
            spackf = cpool.tile([128, NTILES], F32, tag="spackf")
            spackb = cpool.tile([128, NTILES], BF16, tag="spackb")
            spackbf = cpool.tile([128, NTILES], F32, tag="spackbf")
            biasp = cpool.tile([128, NTILES], F32, tag="biasp")
            tpackf = cpool.tile([128, NTILES], F32, tag="tpackf")
            tpackb = cpool.tile([128, NTILES], BF16, tag="tpackb")
            s1acc = cpool.tile([128, NTILES], F32, tag="s1acc")
            wsb = cpool.tile([65, 3 * CW], F32, tag="wsb")

            # ---- input DMAs --------------------------------------------------
            for s in range(16):
                sl = slice(s * 512, (s + 1) * 512)
                nc.sync.dma_start(out=xtsb[:, sl], in_=xt_d[:, sl])
            for s in range(9):
                sl = slice(s * 512, (s + 1) * 512)
                nc.sync.dma_start(out=xlpsb[:, sl], in_=xlp_d[:, sl])
            nc.gpsimd.dma_start(out=psb[:, :], in_=_ap(params_d, [[1, D], [0, 1]]))
            nc.gpsimd.dma_start(out=tpackf[:, :], in_=tp_d[:, :])

            # ---- small setup -------------------------------------------------
            nc.vector.tensor_scalar_mul(rp2[:, :], psb[:, :], 2.0)
            nc.scalar.activation(out=rp[:, :], in_=rp2[:, :],
                                 func=mybir.ActivationFunctionType.Sqrt)
            nc.gpsimd.memset(negh[:, :], -0.5)
            nc.gpsimd.memset(XSLp[D : D + 1, :], 1.0)
            nc.gpsimd.memset(wsb[:, :], 0.0)
            nc.vector.tensor_copy(out=tpackb[:, :], in_=tpackf[:, :])

            # ---- casts (split DVE/ACT per slice) -----------------------------
            # XSR rows 0..63 = bf16(xtsb * rp); zz = XSR^2
            # XSLp rows 0..63 = bf16(xlpsb * rp); zp = XSLp^2
            cast_jobs = []
            for s in range(8):
                sl = slice(s * 1024, (s + 1) * 1024)
                cast_jobs.append(("mulcast", XSR, xtsb, sl))
            for s in range(5):
                sl = slice(s * 1024, min((s + 1) * 1024, PK))
                cast_jobs.append(("mulcast", XSLp, xlpsb, sl))
            for i, (kind, dst, src, sl) in enumerate(cast_jobs):
                if i % 2 == 0:
                    nc.vector.tensor_scalar_mul(dst[0:D, sl], src[:, sl], rp[:, :])
                else:
                    nc.scalar.activation(out=dst[0:D, sl], in_=src[:, sl],
                                         func=mybir.ActivationFunctionType.Copy,
                                         scale=rp[:, :])
            sq_jobs = []
            for s in range(8):
                sl = slice(s * 1024, (s + 1) * 1024)
                sq_jobs.append((zz, XSR, sl))
            for s in range(5):
                sl = slice(s * 1024, min((s + 1) * 1024, PK))
                sq_jobs.append((zp, XSLp, sl))
            for i, (dst, src, sl) in enumerate(sq_jobs):
                if i % 2 == 1:
                    nc.vector.tensor_mul(dst[:, sl], src[0:D, sl], src[0:D, sl])
                else:
                    nc.scalar.activation(out=dst[:, sl], in_=src[0:D, sl],
                                         func=mybir.ActivationFunctionType.Square)

            # ---- PE reduces: s = -0.5 * colsum(z) ----------------------------
            for s in range(16):
                sl = slice(s * 512, (s + 1) * 512)
                q = qpool.tile([1, 512], F32, tag="qps")
                nc.tensor.matmul(q[0:1, :], negh[:, :], zz[:, sl], start=True, stop=True)
                if s % 2 == 0:
                    nc.scalar.copy(out=srowrowf[:, sl], in_=q[0:1, :])
                else:
                    nc.vector.tensor_copy(out=srowrowf[:, sl], in_=q[0:1, :])
            for s in range(9):
                sl = slice(s * 512, (s + 1) * 512)
                q = qpool.tile([1, 512], F32, tag="qps")
                nc.tensor.matmul(q[0:1, :], negh[:, :], zp[:, sl], start=True, stop=True)
                if s % 2 == 1:
                    nc.scalar.copy(out=spackrow[:, sl], in_=q[0:1, :])
                else:
                    nc.vector.tensor_copy(out=spackrow[:, sl], in_=q[0:1, :])

            # ---- srow (global, bf16) via DRAM bounce -> XSR row 64 -----------
            nc.gpsimd.dma_start(out=_ap(srowf_d, [[0, 1], [1, N]]), in_=srowrowf[:, :])
            nc.gpsimd.dma_start(out=s128f[:, :], in_=_ap(srowf_d, [[1, 128], [128, N // 128]]))
            nc.vector.tensor_copy(out=srow128b[:, :], in_=s128f[:, :])
            nc.gpsimd.dma_start(out=_ap(srowb_d, [[1, 128], [128, N // 128]]), in_=srow128b[:, :])
            nc.gpsimd.dma_start(out=XSR[D : D + 1, :], in_=_ap(srowb_d, [[0, 1], [1, N]]))

            # ---- slot-packed bias via DRAM bounce ----------------------------
            # biasp[:, i] = 2*s_pack - f32(bf16(s_pack))  (= -(T + srow) at rb_i)
            nc.gpsimd.dma_start(out=_ap(spackf_d, [[0, 1], [1, PK]]), in_=spackrow[:, :])
            nc.gpsimd.dma_start(out=spackf[:, :], in_=_ap(spackf_d, [[1, 128], [128, NTILES]]))
            nc.vector.tensor_copy(out=spackb[:, :], in_=spackf[:, :])
            nc.vector.tensor_copy(out=spackbf[:, :], in_=spackb[:, :])
            nc.vector.scalar_tensor_tensor(
                out=biasp[:, :], in0=spackf[:, :], scalar=2.0, in1=spackbf[:, :],
                op0=mybir.AluOpType.mult, op1=mybir.AluOpType.subtract,
            )

            # ---- main loop ---------------------------------------------------
            # w strips: ct group g accumulates in psum tile (g//3) at row
            # 32*(g%3); strips drained per psum tile (after ct 2, 5, 7).
            wtiles = {}
            for i in range(NTILES):
                ct = SLOT_CT[i]
                first = i == 0 or SLOT_CT[i - 1] != ct
                last = i == NTILES - 1 or SLOT_CT[i + 1] != ct
                k, row = ct // 3, 32 * (ct % 3)
                if first and row == 0:
                    nparts = 65 if k < 2 else 34
                    wtiles[k] = wpool.tile(
                        [nparts, CW], F32, tag="wps", name=f"wt{k}", bufs=1
                    )
                wt = wtiles[k]
                lhsT = XSLp[:, i * 128 : (i + 1) * 128]
                mm = mpool.tile([128, CW], F32, tag="mm")
                for j in range(CW // 512):
                    sl = slice(ct * CW + j * 512, ct * CW + (j + 1) * 512)
                    nc.tensor.matmul(
                        mm[:, j * 512 : (j + 1) * 512], lhsT, XSR[:, sl],
                        start=True, stop=True,
                    )
                E = epool.tile([128, CW], BF16, tag="E")
                nc.scalar.activation(
                    out=E[:, :], in_=mm[:, :],
                    func=mybir.ActivationFunctionType.Exp,
                    bias=biasp[:, i : i + 1], scale=1.0,
                )
                sc1 = spool.tile([128, CW], BF16, tag="sc1")
                nc.vector.scalar_tensor_tensor(
                    out=sc1[:, :], in0=E[:, :], scalar=1.0, in1=E[:, :],
                    op0=mybir.AluOpType.mult, op1=mybir.AluOpType.mult,
                    accum_out=s1acc[:, i : i + 1],
                )
                for j in range(CW // 512):
                    nc.tensor.matmul(
                        wt[row : row + 1, j * 512 : (j + 1) * 512],
                        tpackb[:, i : i + 1],
                        E[:, j * 512 : (j + 1) * 512],
                        start=first, stop=last,
                    )
                if last and (ct % 3 == 2 or ct == NST - 1):
                    npp = 65 if k < 2 else 34
                    nc.scalar.copy(out=wsb[0:npp, k * CW : (k + 1) * CW], in_=wt[:, :])

            nc.sync.dma_start(out=s1o_d[:, :], in_=s1acc[:, :])
            nc.sync.dma_start(out=wo_d[:, :], in_=wsb[:, :])

    nc.compile()
    return nc


_NC_CACHE = None


def make_in_maps(X, target, params):
    X = np.ascontiguousarray(X, dtype=np.float32)
    target = np.ascontiguousarray(target, dtype=np.float32)
    params = np.ascontiguousarray(params, dtype=np.float32)
    xt = np.ascontiguousarray(X.T)
    maps = []
    for c in range(NCORES):
        rbs = slot_rbs(c)
        wgt = slot_weights(c)
        xlp = np.concatenate([xt[:, rb * 128 : (rb + 1) * 128] for rb in rbs], axis=1)
        tp = np.stack(
            [w * target[rb * 128 : (rb + 1) * 128] for rb, w in zip(rbs, wgt)], axis=1
        )
        maps.append({
            "xt": xt,
            "xlp": np.ascontiguousarray(xlp),
            "tp": np.ascontiguousarray(tp.astype(np.float32)),
            "params": params,
        })
    return maps


def kernel(X, target, params):
    global _NC_CACHE
    X = np.ascontiguousarray(X, dtype=np.float32)
    target = np.ascontiguousarray(target, dtype=np.float32)
    params = np.ascontiguousarray(params, dtype=np.float32)

    in_maps = make_in_maps(X, target, params)

    if _NC_CACHE is None:
        _NC_CACHE = build_kernel()
    res = run_bass_kernel_spmd(_NC_CACHE, in_maps, core_ids=list(range(NCORES)))

    s1 = 0.0
    s2 = 0.0
    t64 = target.astype(np.float64)
    for c in range(NCORES):
        wgt = slot_weights(c)
        s1o = res.results[c]["s1o"].astype(np.float64)  # [128, NTILES]
        wo = res.results[c]["wo"].astype(np.float64)    # [65, 3*CW]
        for i in range(NTILES):
            s1 += wgt[i] * float(s1o[:, i].sum())
        for ct in range(NST):
            k, row = ct // 3, 32 * (ct % 3)
            s2 += float(np.dot(wo[row, k * CW : (k + 1) * CW],
                               t64[ct * CW : (ct + 1) * CW]))

    val = -s2 / (N * np.sqrt(s1))
    return np.array(val, dtype=np.float32)


# revision 13
# speedup vs baseline: 1.8060x; 1.0188x over previous
"""Kernel-target-alignment loss on 8 TRN2 NeuronCores (v2: symmetric + bf16).

Math: Xs = X*sqrt(params); d2_ij = ||Xs_i - Xs_j||^2; K = exp(-d2) (diag == 1);
kta = sum(K*tt^T) / (N*sqrt(sum(K*K)));  return -kta.

Design:
  * K is symmetric: compute the 8 diagonal [1024,1024] supertiles (weight 1)
    and the 28 strictly-upper supertiles (weight 2) -- 36/64 of the work.
    Tiles are [128, 1024].  Tiles with column supertile ct=c exist for global
    row blocks rb in [0, 8(c+1)): 8(c+1) of them, so every core takes the
    (c+1) tiles with rb === core (mod 8).  This makes the slot sequence
    (ct pattern, 36 slots) IDENTICAL on every core -- one SPMD program; all
    per-core variation lives in host-packed inputs:
      xlpack [64, 36*128] = xt columns of the core's row block per slot
      tpack  [128, 36]    = target block per slot
  * A = -d2 via one bf16 augmented matmul: lhsT = [XSLp; ones] (K=65),
    rhs = [Xsb; srow] with Xsb = bf16(sqrt(2p)*x), srow_j = bf16(-sq_j) where
    sq comes from the same bf16 products (PE reduce of z = Xsb*Xsb), so the
    ACT exp bias b = -(T + srow) (fp32, from a slot-packed reduce of
    zp = XSLp*XSLp) cancels the diagonal to ~1e-2: bf16 exp(A_ii) == 1 +- 1%.
  * E = exp(A + b) bf16 on ACT.  s1 partial: DVE scalar_tensor_tensor E*E
    with accum -> s1acc[:, slot].  s2 partial: two M=1 PE matmuls
    tpack_slot^T @ E -> PSUM row slot of a [36, 1024] strip, drained once.
  * Host: s1 = sum_slots wgt * sum(s1o[:, slot]);
    s2 = sum_slots wgt * dot(wo[slot], t[ct*1024 : +1024]);
    return -s2 / (N * sqrt(s1)).
"""

import numpy as np

import concourse.bass as bass
import concourse.bacc as bacc
import concourse.tile as tile
import concourse.mybir as mybir
from concourse.bass_utils import run_bass_kernel_spmd

N = 8192
D = 64
NCORES = 8
CW = 1024                  # column tile width
NST = 8                    # supertile grid
NTILES = 36                # tiles (slots) per core
PK = NTILES * 128          # packed lhs columns = 4608

F32 = mybir.dt.float32
BF16 = mybir.dt.bfloat16

# slot pattern: ct per slot, same on every core
SLOT_CT = [c for c in range(NST) for _ in range(c + 1)]
assert len(SLOT_CT) == NTILES


def slot_rbs(core):
    """Global row block per slot for this core."""
    rbs = []
    for c in range(NST):
        for j in range(c + 1):
            rbs.append(8 * j + core)
    return rbs


def slot_weights(core):
    w = []
    for c in range(NST):
        for j in range(c + 1):
            rb = 8 * j + core
            w.append(1.0 if 8 * c <= rb < 8 * (c + 1) else 2.0)
    return w


def _ap(tensor, ap, offset=0):
    return bass.AP(tensor=tensor, offset=offset, ap=ap)


def build_kernel():
    nc = bacc.Bacc("TRN2", target_bir_lowering=False)

    xt_d = nc.dram_tensor("xt", [D, N], F32, kind="ExternalInput")
    xlp_d = nc.dram_tensor("xlp", [D, PK], F32, kind="ExternalInput")
    tp_d = nc.dram_tensor("tp", [128, NTILES], F32, kind="ExternalInput")
    params_d = nc.dram_tensor("params", [D], F32, kind="ExternalInput")
    srowf_d = nc.dram_tensor("srowf_scratch", [N], F32)
    srowb_d = nc.dram_tensor("srowb_scratch", [N], BF16)
    spackf_d = nc.dram_tensor("spackf_scratch", [PK], F32)
    s1o_d = nc.dram_tensor("s1o", [128, NTILES], F32, kind="ExternalOutput")
    wo_d = nc.dram_tensor("wo", [65, 3 * CW], F32, kind="ExternalOutput")

    with tile.TileContext(nc) as tc:
        with (
            tc.tile_pool(name="const", bufs=1) as cpool,
            tc.tile_pool(name="etile", bufs=4) as epool,
            tc.tile_pool(name="scratch", bufs=3) as spool,
            tc.tile_pool(name="mmpsum", bufs=2, space="PSUM") as mpool,
            tc.tile_pool(name="wq", bufs=2, space="PSUM") as wpool,
        ):
            qpool = wpool
            # ---- persistent SBUF tensors -------------------------------------
            xtsb = cpool.tile([D, N], F32, tag="xtsb")
            xlpsb = cpool.tile([D, PK], F32, tag="xlpsb")
            XSR = cpool.tile([D + 1, N], BF16, tag="XSR")    # [Xsb; srow]
            XSLp = cpool.tile([D + 1, PK], BF16, tag="XSLp")  # [Xsb-pack; ones]
            zz = cpool.tile([D, N], BF16, tag="zz")
            zp = cpool.tile([D, PK], BF16, tag="zp")
            psb = cpool.tile([D, 1], F32, tag="psb")
            rp2 = cpool.tile([D, 1], F32, tag="rp2")
            rp = cpool.tile([D, 1], F32, tag="rp")
            negh = cpool.tile([D, 1], BF16, tag="negh")
            srowrowf = cpool.tile([1, N], F32, tag="srowrowf")
            spackrow = cpool.tile([1, PK], F32, tag="spackrow")
            s128f = cpool.tile([128, N // 128], F32, tag="s128f")
            srow128b = cpool.tile([128, N // 128], BF16, tag="srow128b")
            spackf = cpool.tile([128, NTILES], F32, tag="spackf")
            spackb = cpool.tile([128, NTILES], BF16, tag="spackb")
            spackbf = cpool.tile([128, NTILES], F32, tag="spackbf")
            biasp = cpool.tile([128, NTILES], F32, tag="biasp")
            tpackf = cpool.tile([128, NTILES], F32, tag="tpackf")
            tpackb = cpool.tile([128, NTILES], BF16, tag="tpackb")
            s1acc = cpool.tile([128, NTILES], F32, tag="s1acc")
            wsb = cpool.tile([65, 3 * CW], F32, tag="wsb")

            # ---- input DMAs --------------------------------------------------
            for s in range(16):
                sl = slice(s * 512, (s + 1) * 512)
                nc.sync.dma_start(out=xtsb[:, sl], in_=xt_d[:, sl])
            for s in range(9):
                sl = slice(s * 512, (s + 1) * 512)
                nc.sync.dma_start(out=xlpsb[:, sl], in_=xlp_d[:, sl])
            nc.gpsimd.dma_start(out=psb[:, :], in_=_ap(params_d, [[1, D], [0, 1]]))
            nc.gpsimd.dma_start(out=tpackf[:, :], in_=tp_d[:, :])

            # ---- small setup -------------------------------------------------
            nc.vector.tensor_scalar_mul(rp2[:, :], psb[:, :], 2.0)
            nc.scalar.activation(out=rp[:, :], in_=rp2[:, :],
                                 func=mybir.ActivationFunctionType.Sqrt)
            nc.gpsimd.memset(negh[:, :], -0.5)
            nc.gpsimd.memset(XSLp[D : D + 1, :], 1.0)
            nc.gpsimd.memset(wsb[:, :], 0.0)
            nc.vector.tensor_copy(out=tpackb[:, :], in_=tpackf[:, :])

            # ---- casts (split DVE/ACT per slice) -----------------------------
            # XSR rows 0..63 = bf16(xtsb * rp); zz = XSR^2
            # XSLp rows 0..63 = bf16(xlpsb * rp); zp = XSLp^2
            cast_jobs = []
            for s in range(8):
                sl = slice(s * 1024, (s + 1) * 1024)
                cast_jobs.append(("mulcast", XSR, xtsb, sl))
            for s in range(5):
                sl = slice(s * 1024, min((s + 1) * 1024, PK))
                cast_jobs.append(("mulcast", XSLp, xlpsb, sl))
            for i, (kind, dst, src, sl) in enumerate(cast_jobs):
                if i % 2 == 0:
                    nc.vector.tensor_scalar_mul(dst[0:D, sl], src[:, sl], rp[:, :])
                else:
                    nc.scalar.activation(out=dst[0:D, sl], in_=src[:, sl],
                                         func=mybir.ActivationFunctionType.Copy,
                                         scale=rp[:, :])
            sq_jobs = []
            for s in range(8):
                sl = slice(s * 1024, (s + 1) * 1024)
                sq_jobs.append((zz, XSR, sl))
            for s in range(5):
                sl = slice(s * 1024, min((s + 1) * 1024, PK))
                sq_jobs.append((zp, XSLp, sl))
            for i, (dst, src, sl) in enumerate(sq_jobs):
                if i % 2 == 1:
                    nc.vector.tensor_mul(dst[:, sl], src[0:D, sl], src[0:D, sl])
                else:
                    nc.scalar.activation(out=dst[:, sl], in_=src[0:D, sl],
                                         func=mybir.ActivationFunctionType.Square)

            # ---- PE reduces: s = -0.5 * colsum(z) ----------------------------
            for s in range(16):
                sl = slice(s * 512, (s + 1) * 512)
                q = qpool.tile([1, 512], F32, tag="qps")
                nc.tensor.matmul(q[0:1, :], negh[:, :], zz[:, sl], start=True, stop=True)
                if s % 2 == 0:
                    nc.scalar.copy(out=srowrowf[:, sl], in_=q[0:1, :])
                else:
                    nc.vector.tensor_copy(out=srowrowf[:, sl], in_=q[0:1, :])
            for s in range(9):
                sl = slice(s * 512, (s + 1) * 512)
                q = qpool.tile([1, 512], F32, tag="qps")
                nc.tensor.matmul(q[0:1, :], negh[:, :], zp[:, sl], start=True, stop=True)
                if s % 2 == 1:
                    nc.scalar.copy(out=spackrow[:, sl], in_=q[0:1, :])
                else:
                    nc.vector.tensor_copy(out=spackrow[:, sl], in_=q[0:1, :])

            # ---- srow (global, bf16) via DRAM bounce -> XSR row 64 -----------
            nc.gpsimd.dma_start(out=_ap(srowf_d, [[0, 1], [1, N]]), in_=srowrowf[:, :])
            nc.gpsimd.dma_start(out=s128f[:, :], in_=_ap(srowf_d, [[1, 128], [128, N // 128]]))
            nc.vector.tensor_copy(out=srow128b[:, :], in_=s128f[:, :])
            nc.gpsimd.dma_start(out=_ap(srowb_d, [[1, 128], [128, N // 128]]), in_=srow128b[:, :])
            nc.gpsimd.dma_start(out=XSR[D : D + 1, :], in_=_ap(srowb_d, [[0, 1], [1, N]]))

            # ---- slot-packed bias via DRAM bounce (sync queue, parallel with
            # the srow chain on the gpsimd queue) -----------------------------
            # biasp[:, i] = 2*s_pack - f32(bf16(s_pack))  (= -(T + srow) at rb_i)
            nc.sync.dma_start(out=_ap(spackf_d, [[0, 1], [1, PK]]), in_=spackrow[:, :])
            nc.sync.dma_start(out=spackf[:, :], in_=_ap(spackf_d, [[1, 128], [128, NTILES]]))
            nc.vector.tensor_copy(out=spackb[:, :], in_=spackf[:, :])
            nc.vector.tensor_copy(out=spackbf[:, :], in_=spackb[:, :])
            nc.vector.scalar_tensor_tensor(
                out=biasp[:, :], in0=spackf[:, :], scalar=2.0, in1=spackbf[:, :],
                op0=mybir.AluOpType.mult, op1=mybir.AluOpType.subtract,
            )

            # ---- main loop (software pipelined) ------------------------------
            # Stage A(i): matmul -> mm(i).  Stage B(i): exp, stt, w-matmul.
            # B(i) is emitted after A(i+1) so the PE never waits on exp(i)
            # before issuing A(i+1) (in-order engine queues).
            # w strips: ct group g accumulates in psum tile (g//3) at row
            # 32*(g%3); strips drained per psum tile (after ct 2, 5, 7).
            wtiles = {}
            mms = {}
            Es = {}

            def stage_a(i):
                ct = SLOT_CT[i]
                first = i == 0 or SLOT_CT[i - 1] != ct
                k = ct // 3
                if first and ct % 3 == 0:
                    nparts = 65 if k < 2 else 34
                    wtiles[k] = wpool.tile(
                        [nparts, CW], F32, tag="wps", name=f"wt{k}", bufs=1
                    )
                lhsT = XSLp[:, i * 128 : (i + 1) * 128]
                mm = mpool.tile([128, CW], F32, tag="mm", name="mm")
                for j in range(CW // 512):
                    sl = slice(ct * CW + j * 512, ct * CW + (j + 1) * 512)
                    nc.tensor.matmul(
                        mm[:, j * 512 : (j + 1) * 512], lhsT, XSR[:, sl],
                        start=True, stop=True,
                    )
                mms[i] = mm

            def stage_b(i):
                ct = SLOT_CT[i]
                first = i == 0 or SLOT_CT[i - 1] != ct
                last = i == NTILES - 1 or SLOT_CT[i + 1] != ct
                k, row = ct // 3, 32 * (ct % 3)
                wt = wtiles[k]
                E = epool.tile([128, CW], BF16, tag="E", name="E")
                nc.scalar.activation(
                    out=E[:, :], in_=mms.pop(i)[:, :],
                    func=mybir.ActivationFunctionType.Exp,
                    bias=biasp[:, i : i + 1], scale=1.0,
                )
                sc1 = spool.tile([128, CW], BF16, tag="sc1", name="sc1")
                nc.vector.scalar_tensor_tensor(
                    out=sc1[:, :], in0=E[:, :], scalar=1.0, in1=E[:, :],
                    op0=mybir.AluOpType.mult, op1=mybir.AluOpType.mult,
                    accum_out=s1acc[:, i : i + 1],
                )
                for j in range(CW // 512):
                    nc.tensor.matmul(
                        wt[row : row + 1, j * 512 : (j + 1) * 512],
                        tpackb[:, i : i + 1],
                        E[:, j * 512 : (j + 1) * 512],
                        start=first, stop=last,
                    )
                if last and (ct % 3 == 2 or ct == NST - 1):
                    npp = 65 if k < 2 else 34
                    nc.scalar.copy(out=wsb[0:npp, k * CW : (k + 1) * CW], in_=wt[:, :])

            stage_a(0)
            for i in range(NTILES):
                if i + 1 < NTILES:
                    stage_a(i + 1)
                stage_b(i)

            nc.sync.dma_start(out=s1o_d[:, :], in_=s1acc[:, :])
            nc.sync.dma_start(out=wo_d[:, :], in_=wsb[:, :])

    nc.compile()
    return nc


_NC_CACHE = None


def make_in_maps(X, target, params):
    X = np.ascontiguousarray(X, dtype=np.float32)
    target = np.ascontiguousarray(target, dtype=np.float32)
    params = np.ascontiguousarray(params, dtype=np.float32)
    xt = np.ascontiguousarray(X.T)
    maps = []
    for c in range(NCORES):
        rbs = slot_rbs(c)
        wgt = slot_weights(c)
        xlp = np.concatenate([xt[:, rb * 128 : (rb + 1) * 128] for rb in rbs], axis=1)
        tp = np.stack(
            [w * target[rb * 128 : (rb + 1) * 128] for rb, w in zip(rbs, wgt)], axis=1
        )
        maps.append({
            "xt": xt,
            "xlp": np.ascontiguousarray(xlp),
            "tp": np.ascontiguousarray(tp.astype(np.float32)),
            "params": params,
        })
    return maps


def kernel(X, target, params):
    global _NC_CACHE
    X = np.ascontiguousarray(X, dtype=np.float32)
    target = np.ascontiguousarray(target, dtype=np.float32)
    params = np.ascontiguousarray(params, dtype=np.float32)

    in_maps = make_in_maps(X, target, params)

    if _NC_CACHE is None:
        _NC_CACHE = build_kernel()
    res = run_bass_kernel_spmd(_NC_CACHE, in_maps, core_ids=list(range(NCORES)))

    s1 = 0.0
    s2 = 0.0
    t64 = target.astype(np.float64)
    for c in range(NCORES):
        wgt = slot_weights(c)
        s1o = res.results[c]["s1o"].astype(np.float64)  # [128, NTILES]
        wo = res.results[c]["wo"].astype(np.float64)    # [65, 3*CW]
        for i in range(NTILES):
            s1 += wgt[i] * float(s1o[:, i].sum())
        for ct in range(NST):
            k, row = ct // 3, 32 * (ct % 3)
            s2 += float(np.dot(wo[row, k * CW : (k + 1) * CW],
                               t64[ct * CW : (ct + 1) * CW]))

    val = -s2 / (N * np.sqrt(s1))
    return np.array(val, dtype=np.float32)


# revision 14
# speedup vs baseline: 2.3147x; 1.2817x over previous
"""Kernel-target-alignment loss on 8 TRN2 NeuronCores (v3: symmetric + bf16).

Math: Xs = X*sqrt(params); d2_ij = ||Xs_i - Xs_j||^2; K = exp(-d2) (diag == 1);
kta = sum(K*tt^T) / (N*sqrt(sum(K*K)));  return -kta.

Design (see git history for the derivation):
  * Symmetry: compute the 8 diagonal [1024,1024] supertiles (weight 1) and
    the 28 strictly-upper supertiles (weight 2) -- 36/64 of the elements.
    Tiles are [128, 1024].  Tiles with column supertile ct=c exist for global
    row blocks rb in [0, 8(c+1)): every core takes the (c+1) of them with
    rb === core (mod 8), so the slot sequence (ct pattern) is IDENTICAL on
    every core -- one SPMD program.  Per-core variation lives in host-packed
    (layout-only) inputs: xlpack = xt columns per slot, tpack = wgt * t block
    per slot.
  * A = -d2 via one bf16 augmented matmul (K=65): lhsT = [Xsb-pack; ones],
    rhs = [Xsb; srow], Xsb = bf16(sqrt(2p)*x), srow_j = bf16(-sq_j) written
    straight from the PE column-reduce of z = Xsb*Xsb (PSUM -> bf16 copies).
    ACT exp bias b_i = 2*s_i - f32(bf16(s_i)) (fp32, from an identically
    computed slot-packed reduce of zp = XSLp^2) cancels the diagonal to ~1e-2
    so bf16 exp(A_ii) == 1 +- 1%.
  * E = exp(A + b) bf16 on ACT.  s1: DVE scalar_tensor_tensor E*E with accum
    per slot.  s2: two M=1 PE matmuls tpack_slot^T @ E accumulated in PSUM
    row 32*(ct%3) of strip tile ct//3, drained per strip.
  * PE HAM note: bf16 matmuls with 128-column weights (FWL) and K<128 do NOT
    count as "busy" for the HAM clock un-throttle -- the PE would stay at
    1.2 GHz forever.  A short burst of K=128 M=1 matmuls at kernel start
    (plus a drip through setup) warms it to 2.4 GHz and it stays warm.
  * Host: s1 = sum_slots wgt * sum(s1o[:, slot]);
    s2 = sum_ct dot(wo[ct], t[ct*1024:+1024]); return -s2 / (N*sqrt(s1)).
"""

import numpy as np

import concourse.bass as bass
import concourse.bacc as bacc
import concourse.tile as tile
import concourse.mybir as mybir
from concourse.bass_utils import run_bass_kernel_spmd

N = 8192
D = 64
NCORES = 8
CW = 1024                  # column tile width
NST = 8                    # supertile grid
NTILES = 36                # tiles (slots) per core
PK = NTILES * 128          # packed lhs columns = 4608

F32 = mybir.dt.float32
BF16 = mybir.dt.bfloat16

# slot pattern: ct per slot, same on every core
SLOT_CT = [c for c in range(NST) for _ in range(c + 1)]
assert len(SLOT_CT) == NTILES


def slot_rbs(core):
    """Global row block per slot for this core."""
    return [8 * j + core for c in range(NST) for j in range(c + 1)]


def slot_weights(core):
    w = []
    for c in range(NST):
        for j in range(c + 1):
            rb = 8 * j + core
            w.append(1.0 if 8 * c <= rb < 8 * (c + 1) else 2.0)
    return w


def _ap(tensor, ap, offset=0):
    return bass.AP(tensor=tensor, offset=offset, ap=ap)


def build_kernel():
    nc = bacc.Bacc("TRN2", target_bir_lowering=False)

    xt_d = nc.dram_tensor("xt", [D, N], F32, kind="ExternalInput")
    xlp_d = nc.dram_tensor("xlp", [D, PK], F32, kind="ExternalInput")
    tp_d = nc.dram_tensor("tp", [128, NTILES], F32, kind="ExternalInput")
    params_d = nc.dram_tensor("params", [D], F32, kind="ExternalInput")
    ident_d = nc.dram_tensor("ident36", [36, 36], F32, kind="ExternalInput")
    spackf_d = nc.dram_tensor("spackf_scratch", [PK], F32)
    s1o_d = nc.dram_tensor("s1o", [128, NTILES], F32, kind="ExternalOutput")
    wo_d = nc.dram_tensor("wo", [NST, CW], F32, kind="ExternalOutput")

    with tile.TileContext(nc) as tc:
        with (
            tc.tile_pool(name="const", bufs=1) as cpool,
            tc.tile_pool(name="etile", bufs=4) as epool,
            tc.tile_pool(name="scratch", bufs=3) as spool,
            tc.tile_pool(name="mmpsum", bufs=2, space="PSUM") as mpool,
            tc.tile_pool(name="wq", bufs=2, space="PSUM") as wpool,
        ):
            qpool = wpool
            # ---- persistent SBUF tensors -------------------------------------
            xtsb = cpool.tile([D, N], F32, tag="xtsb")
            xlpsb = cpool.tile([D, PK], F32, tag="xlpsb")
            XSR = cpool.tile([D + 1, N], BF16, tag="XSR")    # [Xsb; srow]
            XSLp = cpool.tile([D + 1, PK], BF16, tag="XSLp")  # [Xsb-pack; ones]
            zz = cpool.tile([D, N], BF16, tag="zz")
            zp = cpool.tile([D, PK], BF16, tag="zp")
            psb = cpool.tile([D, 1], F32, tag="psb")
            rp2 = cpool.tile([D, 1], F32, tag="rp2")
            rp = cpool.tile([D, 1], F32, tag="rp")
            negh = cpool.tile([D, 1], BF16, tag="negh")
            spackrow = cpool.tile([1, PK], F32, tag="spackrow")
            sp36 = cpool.tile([36, 128], F32, tag="sp36")
            ident = cpool.tile([36, 36], F32, tag="ident")
            spackf = cpool.tile([128, NTILES], F32, tag="spackf")
            spackb = cpool.tile([128, NTILES], BF16, tag="spackb")
            spackbf = cpool.tile([128, NTILES], F32, tag="spackbf")
            biasp = cpool.tile([128, NTILES], F32, tag="biasp")
            tpackf = cpool.tile([128, NTILES], F32, tag="tpackf")
            tpackb = cpool.tile([128, NTILES], BF16, tag="tpackb")
            s1acc = cpool.tile([128, NTILES], F32, tag="s1acc")
            wsb = cpool.tile([65, 3 * CW], F32, tag="wsb")
            wcol = cpool.tile([128, 1], BF16, tag="wcol")
            wrhs = cpool.tile([128, 512], BF16, tag="wrhs")

            # ---- PE warmup (HAM un-throttle: K=128 M=1 matmuls count as
            # busy; our K=65 FWL matmuls do not) ------------------------------
            nc.gpsimd.memset(wcol[:, :], 0.5)
            nc.gpsimd.memset(wrhs[:, :], 0.5)

            def warm(n):
                for _ in range(n):
                    q = qpool.tile([1, 512], F32, tag="qps", name="wq")
                    nc.tensor.matmul(q[0:1, :], wcol[:, :], wrhs[:, :],
                                     start=True, stop=True)

            warm(14)

            # ---- input DMAs --------------------------------------------------
            for s in range(16):
                sl = slice(s * 512, (s + 1) * 512)
                nc.sync.dma_start(out=xtsb[:, sl], in_=xt_d[:, sl])
            for s in range(9):
                sl = slice(s * 512, (s + 1) * 512)
                nc.sync.dma_start(out=xlpsb[:, sl], in_=xlp_d[:, sl])
            nc.gpsimd.dma_start(out=psb[:, :], in_=_ap(params_d, [[1, D], [0, 1]]))
            nc.gpsimd.dma_start(out=tpackf[:, :], in_=tp_d[:, :])
            nc.gpsimd.dma_start(out=ident[:, :], in_=ident_d[:, :])

            # ---- small setup -------------------------------------------------
            nc.vector.tensor_scalar_mul(rp2[:, :], psb[:, :], 2.0)
            nc.scalar.activation(out=rp[:, :], in_=rp2[:, :],
                                 func=mybir.ActivationFunctionType.Sqrt)
            nc.gpsimd.memset(negh[:, :], -0.5)
            nc.gpsimd.memset(XSLp[D : D + 1, :], 1.0)
            nc.vector.tensor_copy(out=tpackb[:, :], in_=tpackf[:, :])

            # ---- casts + squares + PE column reduces, interleaved ------------
            # XSR rows 0..63 = bf16(xtsb * rp); zz = XSR^2;
            # srow slice = bf16(-0.5 * colsum(zz)) -> XSR row 64 directly.
            eng = 0
            for s in range(8):
                sl = slice(s * 1024, (s + 1) * 1024)
                if s % 2 == 0:
                    nc.vector.tensor_scalar_mul(XSR[0:D, sl], xtsb[:, sl], rp[:, :])
                    nc.scalar.activation(out=zz[:, sl], in_=XSR[0:D, sl],
                                         func=mybir.ActivationFunctionType.Square)
                else:
                    nc.scalar.activation(out=XSR[0:D, sl], in_=xtsb[:, sl],
                                         func=mybir.ActivationFunctionType.Copy,
                                         scale=rp[:, :])
                    nc.vector.tensor_mul(zz[:, sl], XSR[0:D, sl], XSR[0:D, sl])
                for j in range(2):
                    ssl = slice(s * 1024 + j * 512, s * 1024 + j * 512 + 512)
                    q = qpool.tile([1, 512], F32, tag="qps", name="q")
                    nc.tensor.matmul(q[0:1, :], negh[:, :], zz[:, ssl],
                                     start=True, stop=True)
                    if eng % 2 == 0:
                        nc.scalar.copy(out=XSR[D : D + 1, ssl], in_=q[0:1, :])
                    else:
                        nc.vector.tensor_copy(out=XSR[D : D + 1, ssl], in_=q[0:1, :])
                    eng += 1
                warm(1)

            # packed side: XSLp casts, zp squares, reduces -> spackrow (f32)
            for s in range(5):
                sl = slice(s * 1024, min((s + 1) * 1024, PK))
                if s % 2 == 1:
                    nc.vector.tensor_scalar_mul(XSLp[0:D, sl], xlpsb[:, sl], rp[:, :])
                    nc.scalar.activation(out=zp[:, sl], in_=XSLp[0:D, sl],
                                         func=mybir.ActivationFunctionType.Square)
                else:
                    nc.scalar.activation(out=XSLp[0:D, sl], in_=xlpsb[:, sl],
                                         func=mybir.ActivationFunctionType.Copy,
                                         scale=rp[:, :])
                    nc.vector.tensor_mul(zp[:, sl], XSLp[0:D, sl], XSLp[0:D, sl])
                nsub = (min((s + 1) * 1024, PK) - s * 1024) // 512
                for j in range(nsub):
                    ssl = slice(s * 1024 + j * 512, s * 1024 + j * 512 + 512)
                    q = qpool.tile([1, 512], F32, tag="qps", name="q")
                    nc.tensor.matmul(q[0:1, :], negh[:, :], zp[:, ssl],
                                     start=True, stop=True)
                    if eng % 2 == 0:
                        nc.scalar.copy(out=spackrow[:, ssl], in_=q[0:1, :])
                    else:
                        nc.vector.tensor_copy(out=spackrow[:, ssl], in_=q[0:1, :])
                    eng += 1
                warm(1)

            # ---- slot-packed bias: contiguous bounce + PE transpose ----------
            # biasp[:, i] = 2*s_i - f32(bf16(s_i))   (= -(T + srow) at rb_i)
            nc.sync.dma_start(out=_ap(spackf_d, [[0, 1], [1, PK]]), in_=spackrow[:, :])
            warm(2)
            nc.sync.dma_start(out=sp36[:, :], in_=_ap(spackf_d, [[128, 36], [1, 128]]))
            warm(2)
            qt = qpool.tile([128, 36], F32, tag="qps", name="qt")
            nc.tensor.transpose(qt[:, :], sp36[:, :], ident[:, :])
            nc.vector.tensor_copy(out=spackf[:, :], in_=qt[:, :])
            nc.vector.tensor_copy(out=spackb[:, :], in_=spackf[:, :])
            nc.vector.tensor_copy(out=spackbf[:, :], in_=spackb[:, :])
            nc.vector.scalar_tensor_tensor(
                out=biasp[:, :], in0=spackf[:, :], scalar=2.0, in1=spackbf[:, :],
                op0=mybir.AluOpType.mult, op1=mybir.AluOpType.subtract,
            )
            warm(2)

            # ---- main loop (software pipelined) ------------------------------
            wtiles = {}
            mms = {}

            def stage_a(i):
                ct = SLOT_CT[i]
                first = i == 0 or SLOT_CT[i - 1] != ct
                k = ct // 3
                if first and ct % 3 == 0:
                    nparts = 65 if k < 2 else 34
                    wtiles[k] = wpool.tile(
                        [nparts, CW], F32, tag="wps", name=f"wt{k}", bufs=1
                    )
                lhsT = XSLp[:, i * 128 : (i + 1) * 128]
                mm = mpool.tile([128, CW], F32, tag="mm", name="mm")
                for j in range(CW // 512):
                    sl = slice(ct * CW + j * 512, ct * CW + (j + 1) * 512)
                    nc.tensor.matmul(
                        mm[:, j * 512 : (j + 1) * 512], lhsT, XSR[:, sl],
                        start=True, stop=True,
                    )
                mms[i] = mm

            def stage_b(i):
                ct = SLOT_CT[i]
                first = i == 0 or SLOT_CT[i - 1] != ct
                last = i == NTILES - 1 or SLOT_CT[i + 1] != ct
                k, row = ct // 3, 32 * (ct % 3)
                wt = wtiles[k]
                E = epool.tile([128, CW], BF16, tag="E", name="E")
                nc.scalar.activation(
                    out=E[:, :], in_=mms.pop(i)[:, :],
                    func=mybir.ActivationFunctionType.Exp,
                    bias=biasp[:, i : i + 1], scale=1.0,
                )
                sc1 = spool.tile([128, CW], BF16, tag="sc1", name="sc1")
                nc.vector.scalar_tensor_tensor(
                    out=sc1[:, :], in0=E[:, :], scalar=1.0, in1=E[:, :],
                    op0=mybir.AluOpType.mult, op1=mybir.AluOpType.mult,
                    accum_out=s1acc[:, i : i + 1],
                )
                for j in range(CW // 512):
                    nc.tensor.matmul(
                        wt[row : row + 1, j * 512 : (j + 1) * 512],
                        tpackb[:, i : i + 1],
                        E[:, j * 512 : (j + 1) * 512],
                        start=first, stop=last,
                    )
                if last and (ct % 3 == 2 or ct == NST - 1):
                    npp = 65 if k < 2 else 34
                    if k == 1:
                        nc.vector.tensor_copy(
                            out=wsb[0:npp, k * CW : (k + 1) * CW], in_=wt[:, :])
                    else:
                        nc.scalar.copy(
                            out=wsb[0:npp, k * CW : (k + 1) * CW], in_=wt[:, :])

            stage_a(0)
            for i in range(NTILES):
                if i + 1 < NTILES:
                    stage_a(i + 1)
                stage_b(i)

            nc.sync.dma_start(out=s1o_d[:, :], in_=s1acc[:, :])
            for ct in range(NST):
                k, row = ct // 3, 32 * (ct % 3)
                nc.sync.dma_start(
                    out=wo_d[ct : ct + 1, :],
                    in_=wsb[row : row + 1, k * CW : (k + 1) * CW],
                )

    nc.compile()
    return nc


_NC_CACHE = None


def make_in_maps(X, target, params):
    X = np.ascontiguousarray(X, dtype=np.float32)
    target = np.ascontiguousarray(target, dtype=np.float32)
    params = np.ascontiguousarray(params, dtype=np.float32)
    xt = np.ascontiguousarray(X.T)
    ident = np.eye(36, dtype=np.float32)
    maps = []
    for c in range(NCORES):
        rbs = slot_rbs(c)
        wgt = slot_weights(c)
        xlp = np.concatenate([xt[:, rb * 128 : (rb + 1) * 128] for rb in rbs], axis=1)
        tp = np.stack(
            [w * target[rb * 128 : (rb + 1) * 128] for rb, w in zip(rbs, wgt)], axis=1
        )
        maps.append({
            "xt": xt,
            "xlp": np.ascontiguousarray(xlp),
            "tp": np.ascontiguousarray(tp.astype(np.float32)),
            "params": params,
            "ident36": ident,
        })
    return maps


def kernel(X, target, params):
    global _NC_CACHE
    X = np.ascontiguousarray(X, dtype=np.float32)
    target = np.ascontiguousarray(target, dtype=np.float32)
    params = np.ascontiguousarray(params, dtype=np.float32)

    in_maps = make_in_maps(X, target, params)

    if _NC_CACHE is None:
        _NC_CACHE = build_kernel()
    res = run_bass_kernel_spmd(_NC_CACHE, in_maps, core_ids=list(range(NCORES)))

    s1 = 0.0
    s2 = 0.0
    t64 = target.astype(np.float64)
    for c in range(NCORES):
        wgt = slot_weights(c)
        s1o = res.results[c]["s1o"].astype(np.float64)  # [128, NTILES]
        wo = res.results[c]["wo"].astype(np.float64)    # [NST, CW]
        for i in range(NTILES):
            s1 += wgt[i] * float(s1o[:, i].sum())
        for ct in range(NST):
            s2 += float(np.dot(wo[ct], t64[ct * CW : (ct + 1) * CW]))

    val = -s2 / (N * np.sqrt(s1))
    return np.array(val, dtype=np.float32)
